# revision 18
# baseline (speedup 1.0000x reference)
"""GAT encoder (3-layer) on 8 Trainium2 NeuronCores.

Sharding: destination nodes split contiguously across the 8 cores (each core
owns R nodes = R/128 windows of 128 dst nodes). Per layer:

  1. dense phase (sharded): h = x@W for the core's node slice, with
     ld = h@a_dst folded in as an extra matmul column (W@a_dst appended to W).
  2. AllGather of the per-node h table (f16) across cores -> a_full [NPAD,128].
  3. edge phase (fully local per core): for every edge targeting this core,
     ONE dma_gather fetches h[src] rows (256B each). Everything else is
     computed on-chip:
       - ls[src] = h[src]. a_src  via DVE multiply + free-dim reduce
         (a_src replicated across partitions as a constant).
       - the dst one-hot is built with is_equal(iota_row, dst_col) from a
         host-precomputed per-slot dst column (f16), no gather.
       - ld[dst] = reduce(onehot * ld_row) where ld_row is the window's ld
         column transposed+broadcast via two small PE matmuls per window.
       - w = exp(leaky_relu(ls+ld)); ow = onehot * w is the stationary matmul
         operand; rhs is the gathered h with a ones column appended (fat rhs),
         so ONE matmul per tile accumulates numerator (128 cols) and
         denominator (col 128) in PSUM.
  4. out[d] = num/den + bias, transposed and fed to the next layer's dense
     phase on the fly (layer 3 writes the output slice).

Edge slots are padded so the schedule is identical on all 8 cores (SPMD);
pad slots have dst_col=255 -> all-zero one-hot -> contribute exactly 0.
"""
import sys

sys.path.insert(0, "/opt/trn_rl_repo")

import numpy as np

import os
os.environ.setdefault("JAX_COMPILATION_CACHE_DIR", "/tmp/jax_cache")

import concourse.bacc as bacc
import concourse.bass as bass
import concourse.mybir as mybir
import concourse.tile as tile

F16 = mybir.dt.float16
F32 = mybir.dt.float32
I16 = mybir.dt.int16
ALU = mybir.AluOpType
ACTF = mybir.ActivationFunctionType
AXIS = mybir.AxisListType

P = 128
CORES = 8
BS = 32768            # src-block size for int16 gather indices
STRIPE = BS // CORES  # per-core rows contributed to each block table (4096)
NEG_SLOPE = 0.2
SC_WIN = 6            # windows per super-chunk (= live PSUM accumulators)
GQ = (0, 1, 2, 3)     # SWDGE queues for the gather (chunked round-robin)

LAST_RESULTS = None   # results of the most recent run (for test.py)


# ---------------------------------------------------------------- host layout

def build_plan(edge_index, n_real, n_layers):
    """Edge layout. The schedule (super-chunks -> block runs -> tiles) is
    uniform across cores; only the index data differs per core."""
    R = ((n_real + CORES * P - 1) // (CORES * P)) * P       # nodes per core
    NPAD = R * CORES
    NWC = R // P                                            # windows per core

    src = np.asarray(edge_index[0], dtype=np.int64)
    dst = np.asarray(edge_index[1], dtype=np.int64)
    loops = np.arange(NPAD, dtype=np.int64)
    src = np.concatenate([src, loops])
    dst = np.concatenate([dst, loops])

    core = dst // R
    wloc = (dst % R) // P
    # table row of node n: stripe-interleaved so each gather block is its own
    # AllGather output (block b = stripe b of every core, rank-major). The
    # last stripe is short (R % STRIPE).
    sc_, sj_ = src // R, src % R
    trow = (sj_ // STRIPE) * BS + sc_ * np.minimum(STRIPE, R - (sj_ // STRIPE) * STRIPE) + (sj_ % STRIPE)
    blk = sj_ // STRIPE
    NB = int(blk.max()) + 1

    key = (core * NWC + wloc) * NB + blk
    cnt = np.bincount(key, minlength=CORES * NWC * NB).reshape(CORES, NWC, NB)
    twb = -(-cnt.max(axis=0) // P)          # [NWC, NB]: tiles per (w, block)

    scs = []
    slot_ofs = 0
    for w0 in range(0, NWC, SC_WIN):
        ws = list(range(w0, min(w0 + SC_WIN, NWC)))
        sc_ofs = slot_ofs
        runs = []
        for b in range(NB):
            tiles = []
            r_ofs = slot_ofs
            for w in ws:
                nt = int(twb[w, b])
                if nt:
                    tiles.append((w, nt, slot_ofs))
                    slot_ofs += nt * P
            if slot_ofs > r_ofs:
                runs.append(dict(block=b, tiles=tiles, ofs=r_ofs,
                                 nslots=slot_ofs - r_ofs))
        scs.append(dict(windows=ws, runs=runs, ofs=sc_ofs, end=slot_ofs))
    S = slot_ofs

    # fill slots: edges sorted by (core, window, block)
    order = np.lexsort((blk, wloc, core))
    srcs, dsts = src[order], dst[order]
    cores_s, wl_s, bl_s = core[order], wloc[order], blk[order]

    base = np.zeros((NWC, NB), dtype=np.int64)
    for sc in scs:
        for run in sc["runs"]:
            for (w, nt, ofs) in run["tiles"]:
                base[w, run["block"]] = ofs
    grp = (cores_s * NWC + wl_s) * NB + bl_s
    gstart = np.zeros(CORES * NWC * NB + 1, dtype=np.int64)
    np.cumsum(np.bincount(grp, minlength=CORES * NWC * NB), out=gstart[1:])
    within = np.arange(len(srcs)) - gstart[grp]
    slot = base[wl_s, bl_s] + within

    trow_s = trow[order]
    src_loc = (trow_s - bl_s * BS).astype(np.int16)
    assert src_loc.min() >= 0

    src_w = np.zeros((CORES, P, S // 16), np.int16)          # pad -> row 0
    src_w[cores_s, slot % 16, slot // 16] = src_loc
    for g in range(1, 8):
        src_w[:, 16 * g:16 * (g + 1)] = src_w[:, :16]

    # per-slot dst-within-window column, partition-major: slot s -> partition
    # s%128, column s//128. pad slots get 255 (matches no iota value).
    dst_col = np.full((CORES, P, S // P), 255.0, np.float16)
    dst_col[cores_s, slot % P, slot // P] = (dsts % P).astype(np.float16)

    max_rt = max((run["nslots"] // P
                  for sc in scs for run in sc["runs"]), default=1)
    max_sct = max(((sc["end"] - sc["ofs"]) // P for sc in scs), default=1)
    return dict(R=R, NPAD=NPAD, NWC=NWC, NB=NB, S=S, scs=scs,
                max_rt=max_rt, max_sct=max_sct, src_w=src_w, dst_col=dst_col,
                n_layers=n_layers)


# ------------------------------------------------------------ device program

def build_program(plan, with_collectives=True, compile_program=True,
                  skip=frozenset()):
    R, NPAD, NWC, NB, S = (plan[k] for k in ("R", "NPAD", "NWC", "NB", "S"))
    L = plan["n_layers"]
    scs, max_rt = plan["scs"], plan["max_rt"]

    nc = bacc.Bacc("TRN2", target_bir_lowering=False, num_devices=CORES,
                   num_swdge_queues=4)

    x0T_d = nc.dram_tensor("x0T", [P, R], F32, kind="ExternalInput")
    W_d = nc.dram_tensor("W", [L, P, P], F32, kind="ExternalInput")
    wad_d = nc.dram_tensor("Wad", [L, P, 1], F32, kind="ExternalInput")
    asr_d = nc.dram_tensor("asr", [L, P, P], F16, kind="ExternalInput")
    bias_d = nc.dram_tensor("bias_rep", [L, P, P], F32, kind="ExternalInput")
    iota_d = nc.dram_tensor("iota128", [P, P], F16, kind="ExternalInput")
    srcw_d = nc.dram_tensor("src_w", [P, S // 16], I16, kind="ExternalInput")
    dstc_d = nc.dram_tensor("dst_col", [P, S // P], F16, kind="ExternalInput")
    out_d = nc.dram_tensor("out_x", [R, P], F32, kind="ExternalOutput")

    with tile.TileContext(nc) as tc:
        with tc.tile_pool(name="cst", bufs=1) as cst, \
             tc.tile_pool(name="gbuf", bufs=3) as gbuf, \
             tc.tile_pool(name="wbuf", bufs=3) as wbuf, \
             tc.tile_pool(name="scb", bufs=2) as scb, \
             tc.tile_pool(name="pacc", bufs=SC_WIN, space="PSUM") as pacc, \
             tc.tile_pool(name="paux", bufs=1, space="PSUM") as paux, \
             tc.tile_pool(name="ptrp", bufs=1, space="PSUM") as ptrp, \
             tc.tile_pool(name="dram", bufs=1, space="DRAM") as dram:

            from concourse.masks import make_identity
            ident32 = cst.tile([P, P], F32)
            make_identity(nc, ident32[:])
            iota16 = cst.tile([P, P], F16)
            nc.sync.dma_start(iota16[:], iota_d[:])
            ones128 = cst.tile([P, P], F16)
            nc.vector.memset(ones128[:], 1.0)

            waug = []
            bias_sb = []
            asr_sb = []
            for l in range(L):
                wa = cst.tile([P, 129], F32, name=f"waug{l}")
                nc.sync.dma_start(wa[:, 0:P], W_d[l])
                nc.sync.dma_start(wa[:, 128:129], wad_d[l])
                waug.append(wa)
                bb = cst.tile([P, P], F32, name=f"bias{l}")
                nc.sync.dma_start(bb[:], bias_d[l])
                bias_sb.append(bb)
                ar = cst.tile([P, P], F16, name=f"asr{l}")
                nc.sync.dma_start(ar[:], asr_d[l])
                asr_sb.append(ar)

            srcw_sb = cst.tile([P, S // 16], I16)
            nc.sync.dma_start(srcw_sb[:], srcw_d[:])
            dstc_sb = cst.tile([P, S // P], F16)
            nc.sync.dma_start(dstc_sb[:], dstc_d[:])

            # ld columns per window, per layer (dense phase writes, edge reads)
            ld_cols = [cst.tile([P, NWC], F32, name=f"ldc{l}")
                       for l in range(L)]

            a_slice, a_full = [], []
            for l in range(L):
                a_slice.append(dram.tile([R, P], F16, name=f"a_slice{l}",
                                         tag=f"a_slice{l}"))
                blks = []
                for b in range(NB):
                    stripe = min(STRIPE, R - b * STRIPE)
                    blks.append(dram.tile([stripe * CORES, P], F16,
                                          name=f"a_full{l}_{b}",
                                          tag=f"a_full{l}_{b}",
                                          addr_space="Shared"))
                a_full.append(blks)

            eng_alt = [0]
            run_rr = [0]

            def copy_any(dst_ap, src_ap):
                eng_alt[0] ^= 1
                if eng_alt[0]:
                    nc.vector.tensor_copy(dst_ap, src_ap)
                else:
                    nc.scalar.copy(dst_ap, src_ap)

            def dense_window(l, w, xt_ap):
                pd = paux.tile([P, 129], F32, tag="pdense")
                nc.tensor.matmul(pd[:], lhsT=xt_ap, rhs=waug[l][:],
                                 start=True, stop=True)
                hpa = wbuf.tile([P, P], F16, tag="hpa")
                copy_any(hpa[:], pd[:, 0:P])
                nc.sync.dma_start(a_slice[l][w * P:(w + 1) * P, :], hpa[:])
                nc.vector.tensor_copy(ld_cols[l][:, w:w + 1], pd[:, 128:129])

            def finish_window(l, w, pw):
                rec = wbuf.tile([P, 1], F32, tag="rec")
                nc.vector.reciprocal(rec[:], pw[:, 128:129])
                xn = wbuf.tile([P, P], F32, tag="xn")
                nc.vector.scalar_tensor_tensor(
                    out=xn[:], in0=pw[:, 0:P], scalar=rec[:], op0=ALU.mult,
                    in1=bias_sb[l][:], op1=ALU.add)
                if l < L - 1:
                    pt = ptrp.tile([P, P], F32, tag="ptr")
                    nc.tensor.transpose(pt[:], xn[:], ident32[:])
                    xt = wbuf.tile([P, P], F32, tag="xt")
                    copy_any(xt[:], pt[:])
                    dense_window(l + 1, w, xt[:])
                else:
                    nc.sync.dma_start(out_d[w * P:(w + 1) * P, :], xn[:])

            # layer 0 dense from x0
            for w in range(NWC):
                xt = wbuf.tile([P, P], F32, tag="xt")
                nc.sync.dma_start(xt[:], x0T_d[:, w * P:(w + 1) * P])
                dense_window(0, w, xt[:])

            rg = [list(range(CORES))]
            for l in range(L):
                if with_collectives:
                    for b in range(NB):
                        stripe = min(STRIPE, R - b * STRIPE)
                        nc.gpsimd.collective_compute(
                            "AllGather", ALU.bypass, replica_groups=rg,
                            ins=[a_slice[l][b * STRIPE:
                                            b * STRIPE + stripe, :].opt()],
                            outs=[a_full[l][b][:].opt()])

                for sc in scs:
                    if sc["end"] == sc["ofs"]:
                        continue
                    ws = sc["windows"]
                    nw = len(ws)
                    w0 = ws[0]
                    # ld row per window of the superchunk, replicated across
                    # partitions: diag(ld) via per-partition scalar multiply,
                    # then onesT @ diag broadcasts the row to all partitions.
                    ld_rep = scb.tile([P, SC_WIN, P], F16, tag="ldrep")
                    for k, w in enumerate(ws):
                        diag = wbuf.tile([P, P], F16, tag="diag")
                        nc.vector.tensor_scalar(
                            out=diag[:], in0=ident32[:],
                            scalar1=ld_cols[l][:, w:w + 1], scalar2=None,
                            op0=ALU.mult)
                        pb = ptrp.tile([P, P], F32, tag="ptr")
                        nc.tensor.matmul(pb[:], lhsT=ones128[:],
                                         rhs=diag[:], start=True, stop=True)
                        copy_any(ld_rep[:, k, :], pb[:])

                    pws = {}
                    remaining = {}
                    for run in sc["runs"]:
                        for (w, nt, _) in run["tiles"]:
                            remaining[w] = remaining.get(w, 0) + nt
                    win_total = dict(remaining)

                    for run in sc["runs"]:
                        b = run["block"]
                        n = run["nslots"]
                        rt = n // P
                        ofs = run["ofs"]
                        af = a_full[l][b]

                        ge1 = gbuf.tile([P, max_rt, P], F16, tag="ge1")
                        if "gather" in skip:
                            nc.vector.memset(ge1[:, 0:rt, :], 0.125)
                        else:
                            # chunked gather across queues
                            nq = min(len(GQ), rt)
                            c0 = 0
                            for qi in range(nq):
                                ct = (rt + nq - 1 - qi) // nq
                                cn = ct * P
                                cofs = ofs + c0 * P
                                nc.gpsimd.dma_gather(
                                    ge1[:, c0:c0 + ct, :], af[:, :],
                                    srcw_sb[:, cofs // 16:(cofs + cn) // 16],
                                    cn, cn, P, single_packet=False,
                                    queue_num=GQ[qi])
                                c0 += ct

                        # fat rhs: gathered h + ones column (ACT engine)
                        fat = gbuf.tile([P, max_rt, 129], F16, tag="fat")
                        nc.scalar.copy(fat[:, 0:rt, 0:P], ge1[:, 0:rt, :])
                        nc.vector.memset(fat[:, 0:rt, 128:129], 1.0)

                        t0c = ofs // P
                        oh = gbuf.tile([P, max_rt, P], F16, tag="oh")
                        w16 = wbuf.tile([P, max_rt], F16, tag="w16")
                        if "dve" in skip:
                            nc.vector.memset(oh[:, 0:rt, :], 0.0)
                        else:
                            # gather-independent work first: one-hot + ld
                            nc.vector.tensor_tensor(
                                out=oh[:, 0:rt, :],
                                in0=bass.AP(iota16.tensor, iota16[:].offset,
                                            [iota16[:].ap[0], [0, rt],
                                             [1, P]]),
                                in1=bass.AP(dstc_sb.tensor,
                                            dstc_sb[:].offset + t0c,
                                            [dstc_sb[:].ap[0], [1, rt],
                                             [0, P]]),
                                op=ALU.is_equal)

                            # ld per slot: masked reduce of the window ld rows
                            ld2 = gbuf.tile([P, max_rt, P], F16, tag="t1")
                            for (w, nt, tofs) in run["tiles"]:
                                tt = (tofs - ofs) // P
                                k = w - w0
                                nc.vector.tensor_tensor(
                                    out=ld2[:, tt:tt + nt, :],
                                    in0=oh[:, tt:tt + nt, :],
                                    in1=bass.AP(ld_rep.tensor,
                                                ld_rep[:].offset + k * P,
                                                [ld_rep[:].ap[0], [0, nt],
                                                 [1, P]]),
                                    op=ALU.mult)
                            ld_col = wbuf.tile([P, max_rt], F16,
                                               tag="ld_col")
                            with nc.allow_low_precision(
                                    "1-of-128 select: single nonzero"):
                                nc.vector.tensor_reduce(
                                    ld_col[:, 0:rt], ld2[:, 0:rt, :],
                                    axis=AXIS.X, op=ALU.add)

                            # ls = (h . a_src) per slot
                            t1 = gbuf.tile([P, max_rt, P], F16, tag="t1")
                            nc.vector.tensor_tensor(
                                out=t1[:, 0:rt, :], in0=ge1[:, 0:rt, :],
                                in1=bass.AP(asr_sb[l].tensor,
                                            asr_sb[l][:].offset,
                                            [asr_sb[l][:].ap[0], [0, rt],
                                             [1, P]]),
                                op=ALU.mult)
                            e_col = wbuf.tile([P, max_rt], F32, tag="e_col")
                            nc.vector.tensor_reduce(
                                e_col[:, 0:rt], t1[:, 0:rt, :], axis=AXIS.X,
                                op=ALU.add)
                            ecs = e_col[:, 0:rt]
                            nc.vector.tensor_tensor(out=ecs, in0=ecs,
                                                    in1=ld_col[:, 0:rt],
                                                    op=ALU.add)
                            nc.vector.scalar_tensor_tensor(
                                out=ecs, in0=ecs, scalar=NEG_SLOPE,
                                op0=ALU.mult, in1=ecs, op1=ALU.max)
                            nc.scalar.activation(w16[:, 0:rt], ecs,
                                                 ACTF.Exp)

                            # ow = onehot * w  (in place over oh)
                            nc.vector.tensor_tensor(
                                out=oh[:, 0:rt, :], in0=oh[:, 0:rt, :],
                                in1=bass.AP(w16.tensor, w16[:].offset,
                                            [w16[:].ap[0], [1, rt], [0, P]]),
                                op=ALU.mult)


                        for (w, nt, tofs) in run["tiles"]:
                            if w not in pws:
                                pws[w] = pacc.tile([P, 129], F32, tag="pw",
                                                   name=f"pw_{l}_{w}")
                            pw = pws[w]
                            tt = (tofs - ofs) // P
                            for t in range(nt):
                                is_first = remaining[w] == win_total[w]
                                if "mm" not in skip or is_first:
                                    nc.tensor.matmul(
                                        pw[:, 0:129],
                                        lhsT=oh[:, tt + t, :],
                                        rhs=fat[:, tt + t, :],
                                        start=is_first,
                                        stop=(remaining[w] == 1
                                              or "mm" in skip),
                                        skip_group_check=True)
                                remaining[w] -= 1
                                if remaining[w] == 0:
                                    finish_window(l, w, pw)
                                    del pws[w]
    if compile_program:
        nc.compile()
    return nc


# ------------------------------------------------------------------- kernel

_CACHE = {}

N_REAL = 150000
USER_COUNT = 100000
N_LAYERS = 3


def run_plan(plan, x0, W, a_src, a_dst, bias, n_real):
    """Compile (cached) + run the SPMD program for full node features x0."""
    global LAST_RESULTS
    R, NPAD = plan["R"], plan["NPAD"]
    L = plan["n_layers"]

    key = (plan["S"], plan["NPAD"],
           tuple(tuple((run["block"], tuple(run["tiles"]))
                       for run in sc["runs"]) for sc in plan["scs"]))
    nc = _CACHE.get(key)
    if nc is None:
        nc = build_program(plan)
        _CACHE[key] = nc

    x0p = np.zeros((NPAD, P), np.float32)
    x0p[:n_real] = x0
    bias_rep = np.ascontiguousarray(
        np.broadcast_to(bias[:, None, :], (L, P, P)))
    wad = np.ascontiguousarray(np.einsum("lij,lj->li", W, a_dst)[:, :, None])
    asr = np.ascontiguousarray(
        np.broadcast_to(a_src[:, None, :], (L, P, P))).astype(np.float16)
    iota = np.ascontiguousarray(
        np.broadcast_to(np.arange(P, dtype=np.float16), (P, P)))

    in_maps = []
    for c in range(CORES):
        x0T = np.ascontiguousarray(x0p[c * R:(c + 1) * R].T)
        in_maps.append({
            "x0T": x0T, "W": W, "Wad": wad, "asr": asr,
            "bias_rep": bias_rep, "iota128": iota,
            "src_w": plan["src_w"][c], "dst_col": plan["dst_col"][c],
        })

    run_once, time_iters = make_timed_runner(nc, in_maps)
    results = run_once()
    LAST_RESULTS = dict(results=results, time_iters=time_iters)
    x_out = np.concatenate([results[c]["out_x"]
                            for c in range(CORES)], axis=0)[:n_real]
    return x_out


def make_timed_runner(nc, in_maps):
    """jit once (no donation), keep inputs device-resident; returns
    (run_once() -> per-core results, time_iters(n) -> list of wall seconds)."""
    import time

    import jax
    from jax.sharding import Mesh, PartitionSpec
    from jax.experimental.shard_map import shard_map

    from concourse import bass2jax, mybir as mb
    bass2jax.install_neuronx_cc_hook()

    n_cores = len(in_maps)
    partition_name = (nc.partition_id_tensor.name
                      if nc.partition_id_tensor else None)
    in_names, out_names, out_avals, zero_outs = [], [], [], []
    for alloc in nc.m.functions[0].allocations:
        if not isinstance(alloc, mb.MemoryLocationSet):
            continue
        name = alloc.memorylocations[0].name
        if alloc.kind == "ExternalInput":
            if name != partition_name:
                in_names.append(name)
        elif alloc.kind == "ExternalOutput":
            shape = tuple(alloc.tensor_shape)
            dt = mb.dt.np(alloc.dtype)
            out_names.append(name)
            out_avals.append(jax.core.ShapedArray(shape, dt))
            zero_outs.append(np.zeros(shape, dt))
    n_params = len(in_names)
    all_in = list(in_names) + list(out_names)
    if partition_name is not None:
        all_in.append(partition_name)

    def _body(*args):
        operands = list(args)
        if partition_name is not None:
            operands.append(bass2jax.partition_id_tensor())
        outs = bass2jax._bass_exec_p.bind(
            *operands, out_avals=tuple(out_avals), in_names=tuple(all_in),
            out_names=tuple(out_names),
            lowering_input_output_aliases=(),
            sim_require_finite=False, sim_require_nnan=False, nc=nc)
        return tuple(outs)

    devices = jax.devices()[:n_cores]
    mesh = Mesh(np.asarray(devices), ("core",))
    nin = n_params + len(out_names)
    sharded = jax.jit(shard_map(
        _body, mesh=mesh, in_specs=(PartitionSpec("core"),) * nin,
        out_specs=(PartitionSpec("core"),) * len(out_names),
        check_rep=False), keep_unused=True)

    from jax.sharding import NamedSharding
    sh = NamedSharding(mesh, PartitionSpec("core"))
    concat_in = [jax.device_put(
        np.concatenate([np.asarray(in_maps[c][i]) for c in range(n_cores)],
                       axis=0), sh) for i in in_names]
    concat_zero = [jax.device_put(
        np.zeros((n_cores * z.shape[0], *z.shape[1:]), z.dtype), sh)
        for z in zero_outs]

    def run_once():
        outs = sharded(*concat_in, *concat_zero)
        outs = [np.asarray(o) for o in outs]
        return [{name: outs[i].reshape(n_cores, *out_avals[i].shape)[c]
                 for i, name in enumerate(out_names)}
                for c in range(n_cores)]

    global _LAST_SHARDED, _LAST_ARGS
    _LAST_SHARDED = sharded
    _LAST_ARGS = tuple(concat_in) + tuple(concat_zero)

    def time_iters(n=5):
        ts = []
        for _ in range(n):
            t0 = time.perf_counter()
            outs = sharded(*concat_in, *concat_zero)
            for o in outs:
                o.block_until_ready()
            ts.append(time.perf_counter() - t0)
        return ts

    return run_once, time_iters


def kernel(edge_index, user, item, user_emb, item_emb, W, a_src, a_dst, bias):
    edge_index = np.asarray(edge_index)
    W = np.asarray(W, dtype=np.float32)
    a_src = np.asarray(a_src, dtype=np.float32)
    a_dst = np.asarray(a_dst, dtype=np.float32)
    bias = np.asarray(bias, dtype=np.float32)
    user = np.asarray(user)
    item = np.asarray(item)
    x0 = np.concatenate([np.asarray(user_emb, dtype=np.float32),
                         np.asarray(item_emb, dtype=np.float32)], axis=0)

    plan = build_plan(edge_index, N_REAL, N_LAYERS)
    x3 = run_plan(plan, x0, W, a_src, a_dst, bias, N_REAL)
    return (np.ascontiguousarray(x3[user]),
            np.ascontiguousarray(x3[USER_COUNT + item]))


# revision 19
# speedup vs baseline: 1.2572x; 1.2572x over previous
"""GAT encoder (3-layer) on 8 Trainium2 NeuronCores.

Sharding: destination nodes split contiguously across the 8 cores (each core
owns R nodes = R/128 windows of 128 dst nodes). Per layer:

  1. dense phase (sharded): h = x@W for the core's node slice, with
     ld = h@a_dst folded in as an extra matmul column (W@a_dst appended to W).
  2. AllGather of the per-node h table (f16) across cores -> a_full [NPAD,128].
  3. edge phase (fully local per core): for every edge targeting this core,
     ONE dma_gather fetches h[src] rows (256B each). Everything else is
     computed on-chip:
       - ls[src] = h[src]. a_src  via DVE multiply + free-dim reduce
         (a_src replicated across partitions as a constant).
       - the dst one-hot is built with is_equal(iota_row, dst_col) from a
         host-precomputed per-slot dst column (f16), no gather.
       - ld[dst] = reduce(onehot * ld_row) where ld_row is the window's ld
         column transposed+broadcast via two small PE matmuls per window.
       - w = exp(leaky_relu(ls+ld)); ow = onehot * w is the stationary matmul
         operand; rhs is the gathered h with a ones column appended (fat rhs),
         so ONE matmul per tile accumulates numerator (128 cols) and
         denominator (col 128) in PSUM.
  4. out[d] = num/den + bias, transposed and fed to the next layer's dense
     phase on the fly (layer 3 writes the output slice).

Edge slots are padded so the schedule is identical on all 8 cores (SPMD);
pad slots have dst_col=255 -> all-zero one-hot -> contribute exactly 0.
"""
import sys

sys.path.insert(0, "/opt/trn_rl_repo")

import numpy as np

import os
os.environ.setdefault("JAX_COMPILATION_CACHE_DIR", "/tmp/jax_cache")

import concourse.bacc as bacc
import concourse.bass as bass
import concourse.mybir as mybir
import concourse.tile as tile

F16 = mybir.dt.float16
F32 = mybir.dt.float32
I16 = mybir.dt.int16
ALU = mybir.AluOpType
ACTF = mybir.ActivationFunctionType
AXIS = mybir.AxisListType

P = 128
CORES = 8
BS = 32768            # src-block size for int16 gather indices
STRIPE = BS // CORES  # per-core rows contributed to each block table (4096)
NEG_SLOPE = 0.2
SC_WIN = 6            # windows per super-chunk (= live PSUM accumulators)
GQ = (0, 1, 2, 3)     # SWDGE queues for the gather (chunked round-robin)

LAST_RESULTS = None   # results of the most recent run (for test.py)


# ---------------------------------------------------------------- host layout

def build_plan(edge_index, n_real, n_layers):
    """Edge layout. The schedule (super-chunks -> block runs -> tiles) is
    uniform across cores; only the index data differs per core."""
    R = ((n_real + CORES * P - 1) // (CORES * P)) * P       # nodes per core
    NPAD = R * CORES
    NWC = R // P                                            # windows per core

    src = np.asarray(edge_index[0], dtype=np.int64)
    dst = np.asarray(edge_index[1], dtype=np.int64)
    loops = np.arange(NPAD, dtype=np.int64)
    src = np.concatenate([src, loops])
    dst = np.concatenate([dst, loops])

    core = dst // R
    wloc = (dst % R) // P
    # table row of node n: stripe-interleaved so each gather block is its own
    # AllGather output (block b = stripe b of every core, rank-major). The
    # last stripe is short (R % STRIPE).
    sc_, sj_ = src // R, src % R
    trow = (sj_ // STRIPE) * BS + sc_ * np.minimum(STRIPE, R - (sj_ // STRIPE) * STRIPE) + (sj_ % STRIPE)
    blk = sj_ // STRIPE
    NB = int(blk.max()) + 1

    key = (core * NWC + wloc) * NB + blk
    cnt = np.bincount(key, minlength=CORES * NWC * NB).reshape(CORES, NWC, NB)
    twb = -(-cnt.max(axis=0) // P)          # [NWC, NB]: tiles per (w, block)

    scs = []
    slot_ofs = 0
    for w0 in range(0, NWC, SC_WIN):
        ws = list(range(w0, min(w0 + SC_WIN, NWC)))
        sc_ofs = slot_ofs
        runs = []
        for b in range(NB):
            tiles = []
            r_ofs = slot_ofs
            for w in ws:
                nt = int(twb[w, b])
                if nt:
                    tiles.append((w, nt, slot_ofs))
                    slot_ofs += nt * P
            if slot_ofs > r_ofs:
                runs.append(dict(block=b, tiles=tiles, ofs=r_ofs,
                                 nslots=slot_ofs - r_ofs))
        scs.append(dict(windows=ws, runs=runs, ofs=sc_ofs, end=slot_ofs))
    S = slot_ofs

    # fill slots: edges sorted by (core, window, block)
    order = np.lexsort((blk, wloc, core))
    srcs, dsts = src[order], dst[order]
    cores_s, wl_s, bl_s = core[order], wloc[order], blk[order]

    base = np.zeros((NWC, NB), dtype=np.int64)
    for sc in scs:
        for run in sc["runs"]:
            for (w, nt, ofs) in run["tiles"]:
                base[w, run["block"]] = ofs
    grp = (cores_s * NWC + wl_s) * NB + bl_s
    gstart = np.zeros(CORES * NWC * NB + 1, dtype=np.int64)
    np.cumsum(np.bincount(grp, minlength=CORES * NWC * NB), out=gstart[1:])
    within = np.arange(len(srcs)) - gstart[grp]
    slot = base[wl_s, bl_s] + within

    trow_s = trow[order]
    src_loc = (trow_s - bl_s * BS).astype(np.int16)
    assert src_loc.min() >= 0

    src_w = np.zeros((CORES, P, S // 16), np.int16)          # pad -> row 0
    src_w[cores_s, slot % 16, slot // 16] = src_loc
    for g in range(1, 8):
        src_w[:, 16 * g:16 * (g + 1)] = src_w[:, :16]

    # per-slot dst-within-window column, partition-major: slot s -> partition
    # s%128, column s//128. pad slots get 255 (matches no iota value).
    dst_col = np.full((CORES, P, S // P), 255.0, np.float16)
    dst_col[cores_s, slot % P, slot // P] = (dsts % P).astype(np.float16)

    max_rt = max((run["nslots"] // P
                  for sc in scs for run in sc["runs"]), default=1)
    max_sct = max(((sc["end"] - sc["ofs"]) // P for sc in scs), default=1)
    return dict(R=R, NPAD=NPAD, NWC=NWC, NB=NB, S=S, scs=scs,
                max_rt=max_rt, max_sct=max_sct, src_w=src_w, dst_col=dst_col,
                n_layers=n_layers)


# ------------------------------------------------------------ device program

def build_program(plan, with_collectives=True, compile_program=True,
                  skip=frozenset()):
    R, NPAD, NWC, NB, S = (plan[k] for k in ("R", "NPAD", "NWC", "NB", "S"))
    L = plan["n_layers"]
    scs, max_rt = plan["scs"], plan["max_rt"]

    nc = bacc.Bacc("TRN2", target_bir_lowering=False, num_devices=CORES,
                   num_swdge_queues=4)

    x0T_d = nc.dram_tensor("x0T", [P, R], F32, kind="ExternalInput")
    W_d = nc.dram_tensor("W", [L, P, P], F32, kind="ExternalInput")
    wad_d = nc.dram_tensor("Wad", [L, P, 1], F32, kind="ExternalInput")
    asr_d = nc.dram_tensor("asr", [L, P, P], F16, kind="ExternalInput")
    bias_d = nc.dram_tensor("bias_rep", [L, P, P], F32, kind="ExternalInput")
    iota_d = nc.dram_tensor("iota128", [P, P], F16, kind="ExternalInput")
    srcw_d = nc.dram_tensor("src_w", [P, S // 16], I16, kind="ExternalInput")
    dstc_d = nc.dram_tensor("dst_col", [P, S // P], F16, kind="ExternalInput")
    out_d = nc.dram_tensor("out_x", [R, P], F32, kind="ExternalOutput")

    with tile.TileContext(nc) as tc:
        with tc.tile_pool(name="cst", bufs=1) as cst, \
             tc.tile_pool(name="gbuf", bufs=3) as gbuf, \
             tc.tile_pool(name="wbuf", bufs=3) as wbuf, \
             tc.tile_pool(name="scb", bufs=2) as scb, \
             tc.tile_pool(name="pacc", bufs=SC_WIN, space="PSUM") as pacc, \
             tc.tile_pool(name="paux", bufs=1, space="PSUM") as paux, \
             tc.tile_pool(name="ptrp", bufs=1, space="PSUM") as ptrp, \
             tc.tile_pool(name="dram", bufs=1, space="DRAM") as dram:

            from concourse.masks import make_identity
            ident32 = cst.tile([P, P], F32)
            make_identity(nc, ident32[:])
            iota16 = cst.tile([P, P], F16)
            nc.sync.dma_start(iota16[:], iota_d[:])
            ones128 = cst.tile([P, P], F16)
            nc.vector.memset(ones128[:], 1.0)

            waug = []
            bias_sb = []
            asr_sb = []
            for l in range(L):
                wa = cst.tile([P, 129], F32, name=f"waug{l}")
                nc.sync.dma_start(wa[:, 0:P], W_d[l])
                nc.sync.dma_start(wa[:, 128:129], wad_d[l])
                waug.append(wa)
                bb = cst.tile([P, P], F32, name=f"bias{l}")
                nc.sync.dma_start(bb[:], bias_d[l])
                bias_sb.append(bb)
                ar = cst.tile([P, P], F16, name=f"asr{l}")
                nc.sync.dma_start(ar[:], asr_d[l])
                asr_sb.append(ar)

            srcw_sb = cst.tile([P, S // 16], I16)
            nc.sync.dma_start(srcw_sb[:], srcw_d[:])
            dstc_sb = cst.tile([P, S // P], F16)
            nc.sync.dma_start(dstc_sb[:], dstc_d[:])

            # ld columns per window, per layer (dense phase writes, edge reads)
            ld_cols = [cst.tile([P, NWC], F32, name=f"ldc{l}")
                       for l in range(L)]

            a_slice, a_full = [], []
            for l in range(L):
                a_slice.append(dram.tile([R, P], F16, name=f"a_slice{l}",
                                         tag=f"a_slice{l}"))
                blks = []
                for b in range(NB):
                    stripe = min(STRIPE, R - b * STRIPE)
                    blks.append(dram.tile([stripe * CORES, P], F16,
                                          name=f"a_full{l}_{b}",
                                          tag=f"a_full{l}_{b}",
                                          addr_space="Shared"))
                a_full.append(blks)

            eng_alt = [0]
            run_rr = [0]

            def copy_any(dst_ap, src_ap):
                eng_alt[0] ^= 1
                if eng_alt[0]:
                    nc.vector.tensor_copy(dst_ap, src_ap)
                else:
                    nc.scalar.copy(dst_ap, src_ap)

            def dense_window(l, w, xt_ap):
                pd = paux.tile([P, 129], F32, tag="pdense")
                nc.tensor.matmul(pd[:], lhsT=xt_ap, rhs=waug[l][:],
                                 start=True, stop=True)
                hpa = wbuf.tile([P, P], F16, tag="hpa")
                copy_any(hpa[:], pd[:, 0:P])
                nc.sync.dma_start(a_slice[l][w * P:(w + 1) * P, :], hpa[:])
                nc.vector.tensor_copy(ld_cols[l][:, w:w + 1], pd[:, 128:129])

            def finish_window(l, w, pw):
                rec = wbuf.tile([P, 1], F32, tag="rec")
                nc.vector.reciprocal(rec[:], pw[:, 128:129])
                xn = wbuf.tile([P, P], F32, tag="xn")
                nc.vector.scalar_tensor_tensor(
                    out=xn[:], in0=pw[:, 0:P], scalar=rec[:], op0=ALU.mult,
                    in1=bias_sb[l][:], op1=ALU.add)
                if l < L - 1:
                    pt = ptrp.tile([P, P], F32, tag="ptr")
                    nc.tensor.transpose(pt[:], xn[:], ident32[:])
                    xt = wbuf.tile([P, P], F32, tag="xt")
                    copy_any(xt[:], pt[:])
                    dense_window(l + 1, w, xt[:])
                else:
                    nc.sync.dma_start(out_d[w * P:(w + 1) * P, :], xn[:])

            # layer 0 dense from x0
            for w in range(NWC):
                xt = wbuf.tile([P, P], F32, tag="xt")
                nc.sync.dma_start(xt[:], x0T_d[:, w * P:(w + 1) * P])
                dense_window(0, w, xt[:])

            rg = [list(range(CORES))]
            for l in range(L):
                if with_collectives:
                    for b in range(NB):
                        stripe = min(STRIPE, R - b * STRIPE)
                        nc.gpsimd.collective_compute(
                            "AllGather", ALU.bypass, replica_groups=rg,
                            ins=[a_slice[l][b * STRIPE:
                                            b * STRIPE + stripe, :].opt()],
                            outs=[a_full[l][b][:].opt()])

                for sc in scs:
                    if sc["end"] == sc["ofs"]:
                        continue
                    ws = sc["windows"]
                    nw = len(ws)
                    w0 = ws[0]
                    # ld row per window of the superchunk, replicated across
                    # partitions: diag(ld) via per-partition scalar multiply,
                    # then onesT @ diag broadcasts the row to all partitions.
                    ld_rep = scb.tile([P, SC_WIN, P], F16, tag="ldrep")
                    for k, w in enumerate(ws):
                        diag = wbuf.tile([P, P], F16, tag="diag")
                        nc.vector.tensor_scalar(
                            out=diag[:], in0=ident32[:],
                            scalar1=ld_cols[l][:, w:w + 1], scalar2=None,
                            op0=ALU.mult)
                        pb = ptrp.tile([P, P], F32, tag="ptr")
                        nc.tensor.matmul(pb[:], lhsT=ones128[:],
                                         rhs=diag[:], start=True, stop=True)
                        copy_any(ld_rep[:, k, :], pb[:])

                    pws = {}
                    remaining = {}
                    for run in sc["runs"]:
                        for (w, nt, _) in run["tiles"]:
                            remaining[w] = remaining.get(w, 0) + nt
                    win_total = dict(remaining)

                    for run in sc["runs"]:
                        b = run["block"]
                        n = run["nslots"]
                        rt = n // P
                        ofs = run["ofs"]
                        af = a_full[l][b]

                        ge1 = gbuf.tile([P, max_rt, P], F16, tag="ge1")
                        if "gather" in skip:
                            nc.vector.memset(ge1[:, 0:rt, :], 0.125)
                        else:
                            # chunked gather across queues
                            nq = min(len(GQ), rt)
                            c0 = 0
                            for qi in range(nq):
                                ct = (rt + nq - 1 - qi) // nq
                                cn = ct * P
                                cofs = ofs + c0 * P
                                nc.gpsimd.dma_gather(
                                    ge1[:, c0:c0 + ct, :], af[:, :],
                                    srcw_sb[:, cofs // 16:(cofs + cn) // 16],
                                    cn, cn, P, single_packet=True,
                                    queue_num=GQ[qi])
                                c0 += ct

                        # fat rhs: gathered h + ones column (ACT engine)
                        fat = gbuf.tile([P, max_rt, 129], F16, tag="fat")
                        nc.scalar.copy(fat[:, 0:rt, 0:P], ge1[:, 0:rt, :])
                        nc.vector.memset(fat[:, 0:rt, 128:129], 1.0)

                        t0c = ofs // P
                        oh = gbuf.tile([P, max_rt, P], F16, tag="oh")
                        w16 = wbuf.tile([P, max_rt], F16, tag="w16")
                        if "dve" in skip:
                            nc.vector.memset(oh[:, 0:rt, :], 0.0)
                        else:
                            # gather-independent work first: one-hot + ld
                            nc.vector.tensor_tensor(
                                out=oh[:, 0:rt, :],
                                in0=bass.AP(iota16.tensor, iota16[:].offset,
                                            [iota16[:].ap[0], [0, rt],
                                             [1, P]]),
                                in1=bass.AP(dstc_sb.tensor,
                                            dstc_sb[:].offset + t0c,
                                            [dstc_sb[:].ap[0], [1, rt],
                                             [0, P]]),
                                op=ALU.is_equal)

                            # ld per slot: masked reduce of the window ld rows
                            ld2 = gbuf.tile([P, max_rt, P], F16, tag="t1")
                            for (w, nt, tofs) in run["tiles"]:
                                tt = (tofs - ofs) // P
                                k = w - w0
                                nc.vector.tensor_tensor(
                                    out=ld2[:, tt:tt + nt, :],
                                    in0=oh[:, tt:tt + nt, :],
                                    in1=bass.AP(ld_rep.tensor,
                                                ld_rep[:].offset + k * P,
                                                [ld_rep[:].ap[0], [0, nt],
                                                 [1, P]]),
                                    op=ALU.mult)
                            ld_col = wbuf.tile([P, max_rt], F16,
                                               tag="ld_col")
                            with nc.allow_low_precision(
                                    "1-of-128 select: single nonzero"):
                                nc.vector.tensor_reduce(
                                    ld_col[:, 0:rt], ld2[:, 0:rt, :],
                                    axis=AXIS.X, op=ALU.add)

                            # ls = (h . a_src) per slot
                            t1 = gbuf.tile([P, max_rt, P], F16, tag="t1")
                            nc.vector.tensor_tensor(
                                out=t1[:, 0:rt, :], in0=ge1[:, 0:rt, :],
                                in1=bass.AP(asr_sb[l].tensor,
                                            asr_sb[l][:].offset,
                                            [asr_sb[l][:].ap[0], [0, rt],
                                             [1, P]]),
                                op=ALU.mult)
                            e_col = wbuf.tile([P, max_rt], F32, tag="e_col")
                            nc.vector.tensor_reduce(
                                e_col[:, 0:rt], t1[:, 0:rt, :], axis=AXIS.X,
                                op=ALU.add)
                            ecs = e_col[:, 0:rt]
                            nc.vector.tensor_tensor(out=ecs, in0=ecs,
                                                    in1=ld_col[:, 0:rt],
                                                    op=ALU.add)
                            nc.vector.scalar_tensor_tensor(
                                out=ecs, in0=ecs, scalar=NEG_SLOPE,
                                op0=ALU.mult, in1=ecs, op1=ALU.max)
                            nc.scalar.activation(w16[:, 0:rt], ecs,
                                                 ACTF.Exp)

                            # ow = onehot * w  (in place over oh)
                            nc.vector.tensor_tensor(
                                out=oh[:, 0:rt, :], in0=oh[:, 0:rt, :],
                                in1=bass.AP(w16.tensor, w16[:].offset,
                                            [w16[:].ap[0], [1, rt], [0, P]]),
                                op=ALU.mult)


                        for (w, nt, tofs) in run["tiles"]:
                            if w not in pws:
                                pws[w] = pacc.tile([P, 129], F32, tag="pw",
                                                   name=f"pw_{l}_{w}")
                            pw = pws[w]
                            tt = (tofs - ofs) // P
                            for t in range(nt):
                                is_first = remaining[w] == win_total[w]
                                if "mm" not in skip or is_first:
                                    nc.tensor.matmul(
                                        pw[:, 0:129],
                                        lhsT=oh[:, tt + t, :],
                                        rhs=fat[:, tt + t, :],
                                        start=is_first,
                                        stop=(remaining[w] == 1
                                              or "mm" in skip),
                                        skip_group_check=True)
                                remaining[w] -= 1
                                if remaining[w] == 0:
                                    finish_window(l, w, pw)
                                    del pws[w]
    if compile_program:
        nc.compile()
    return nc


# ------------------------------------------------------------------- kernel

_CACHE = {}

N_REAL = 150000
USER_COUNT = 100000
N_LAYERS = 3


def run_plan(plan, x0, W, a_src, a_dst, bias, n_real):
    """Compile (cached) + run the SPMD program for full node features x0."""
    global LAST_RESULTS
    R, NPAD = plan["R"], plan["NPAD"]
    L = plan["n_layers"]

    key = (plan["S"], plan["NPAD"],
           tuple(tuple((run["block"], tuple(run["tiles"]))
                       for run in sc["runs"]) for sc in plan["scs"]))
    nc = _CACHE.get(key)
    if nc is None:
        nc = build_program(plan)
        _CACHE[key] = nc

    x0p = np.zeros((NPAD, P), np.float32)
    x0p[:n_real] = x0
    bias_rep = np.ascontiguousarray(
        np.broadcast_to(bias[:, None, :], (L, P, P)))
    wad = np.ascontiguousarray(np.einsum("lij,lj->li", W, a_dst)[:, :, None])
    asr = np.ascontiguousarray(
        np.broadcast_to(a_src[:, None, :], (L, P, P))).astype(np.float16)
    iota = np.ascontiguousarray(
        np.broadcast_to(np.arange(P, dtype=np.float16), (P, P)))

    in_maps = []
    for c in range(CORES):
        x0T = np.ascontiguousarray(x0p[c * R:(c + 1) * R].T)
        in_maps.append({
            "x0T": x0T, "W": W, "Wad": wad, "asr": asr,
            "bias_rep": bias_rep, "iota128": iota,
            "src_w": plan["src_w"][c], "dst_col": plan["dst_col"][c],
        })

    run_once, time_iters = make_timed_runner(nc, in_maps)
    results = run_once()
    LAST_RESULTS = dict(results=results, time_iters=time_iters)
    x_out = np.concatenate([results[c]["out_x"]
                            for c in range(CORES)], axis=0)[:n_real]
    return x_out


def make_timed_runner(nc, in_maps):
    """jit once (no donation), keep inputs device-resident; returns
    (run_once() -> per-core results, time_iters(n) -> list of wall seconds)."""
    import time

    import jax
    from jax.sharding import Mesh, PartitionSpec
    from jax.experimental.shard_map import shard_map

    from concourse import bass2jax, mybir as mb
    bass2jax.install_neuronx_cc_hook()

    n_cores = len(in_maps)
    partition_name = (nc.partition_id_tensor.name
                      if nc.partition_id_tensor else None)
    in_names, out_names, out_avals, zero_outs = [], [], [], []
    for alloc in nc.m.functions[0].allocations:
        if not isinstance(alloc, mb.MemoryLocationSet):
            continue
        name = alloc.memorylocations[0].name
        if alloc.kind == "ExternalInput":
            if name != partition_name:
                in_names.append(name)
        elif alloc.kind == "ExternalOutput":
            shape = tuple(alloc.tensor_shape)
            dt = mb.dt.np(alloc.dtype)
            out_names.append(name)
            out_avals.append(jax.core.ShapedArray(shape, dt))
            zero_outs.append(np.zeros(shape, dt))
    n_params = len(in_names)
    all_in = list(in_names) + list(out_names)
    if partition_name is not None:
        all_in.append(partition_name)

    def _body(*args):
        operands = list(args)
        if partition_name is not None:
            operands.append(bass2jax.partition_id_tensor())
        outs = bass2jax._bass_exec_p.bind(
            *operands, out_avals=tuple(out_avals), in_names=tuple(all_in),
            out_names=tuple(out_names),
            lowering_input_output_aliases=(),
            sim_require_finite=False, sim_require_nnan=False, nc=nc)
        return tuple(outs)

    devices = jax.devices()[:n_cores]
    mesh = Mesh(np.asarray(devices), ("core",))
    nin = n_params + len(out_names)
    sharded = jax.jit(shard_map(
        _body, mesh=mesh, in_specs=(PartitionSpec("core"),) * nin,
        out_specs=(PartitionSpec("core"),) * len(out_names),
        check_rep=False), keep_unused=True)

    from jax.sharding import NamedSharding
    sh = NamedSharding(mesh, PartitionSpec("core"))
    concat_in = [jax.device_put(
        np.concatenate([np.asarray(in_maps[c][i]) for c in range(n_cores)],
                       axis=0), sh) for i in in_names]
    concat_zero = [jax.device_put(
        np.zeros((n_cores * z.shape[0], *z.shape[1:]), z.dtype), sh)
        for z in zero_outs]

    def run_once():
        outs = sharded(*concat_in, *concat_zero)
        outs = [np.asarray(o) for o in outs]
        return [{name: outs[i].reshape(n_cores, *out_avals[i].shape)[c]
                 for i, name in enumerate(out_names)}
                for c in range(n_cores)]

    global _LAST_SHARDED, _LAST_ARGS
    _LAST_SHARDED = sharded
    _LAST_ARGS = tuple(concat_in) + tuple(concat_zero)

    def time_iters(n=5):
        ts = []
        for _ in range(n):
            t0 = time.perf_counter()
            outs = sharded(*concat_in, *concat_zero)
            for o in outs:
                o.block_until_ready()
            ts.append(time.perf_counter() - t0)
        return ts

    return run_once, time_iters


def kernel(edge_index, user, item, user_emb, item_emb, W, a_src, a_dst, bias):
    edge_index = np.asarray(edge_index)
    W = np.asarray(W, dtype=np.float32)
    a_src = np.asarray(a_src, dtype=np.float32)
    a_dst = np.asarray(a_dst, dtype=np.float32)
    bias = np.asarray(bias, dtype=np.float32)
    user = np.asarray(user)
    item = np.asarray(item)
    x0 = np.concatenate([np.asarray(user_emb, dtype=np.float32),
                         np.asarray(item_emb, dtype=np.float32)], axis=0)

    plan = build_plan(edge_index, N_REAL, N_LAYERS)
    x3 = run_plan(plan, x0, W, a_src, a_dst, bias, N_REAL)
    return (np.ascontiguousarray(x3[user]),
            np.ascontiguousarray(x3[USER_COUNT + item]))
